# revision 1
# baseline (speedup 1.0000x reference)
"""Batched CG solve on 8 Trainium2 NeuronCores.

Problem: for each of B=256 batches, run `maxiter` conjugate-gradient
iterations on A x = b (A: [1024,1024] SPD, well-conditioned), starting
from x0 = u.reshape(B, 1024).

Strategy (per core, 32 batches, data-parallel over B):
  - Process batches in 8 lockstep groups of G=4.
  - Each group's four A matrices (4 MiB each) are DMA'd into SBUF once and
    all 21 matvecs (1 init + maxiter) read them from SBUF -> HBM traffic is
    1 pass over A instead of 21.
  - Batch g of a group lives on partition 32*g of [128, N] state tiles
    (engine partition-base must be 32-aligned on TRN2). Unused partitions
    carry harmless zeros.
  - Matvec: Ap^T = p^T A (A symmetric). p is PE-transposed to k-major
    weights; fp32r matmuls stream A from SBUF at ~1 elem/cycle.
  - CG scalar updates are fused DVE ops (scalar_tensor_tensor) with
    per-partition scalars; ||r||^2 via ScalarE Square+accumulate.
"""
import sys

sys.path.insert(0, "/opt/trn_rl_repo")

import numpy as np

B, N, GRID = 256, 1024, 32
NCORES = 8
PER_CORE = B // NCORES  # 32
G = 4                   # batches per lockstep group
NGROUPS = PER_CORE // G # 8
KB = N // 128           # 8 k-blocks

_compiled_cache = {}


def _build(maxiter: int):
    import concourse.bacc as bacc
    import concourse.mybir as mybir
    from concourse import tile, masks

    f32 = mybir.dt.float32
    f32r = mybir.dt.float32r
    AT = mybir.AluOpType

    nc = bacc.Bacc()
    A_in = nc.declare_dram_parameter("A", [PER_CORE, N, N], f32r, isOutput=False)
    u_in = nc.declare_dram_parameter("u", [PER_CORE, N], f32, isOutput=False)
    b_in = nc.declare_dram_parameter("b", [PER_CORE, N], f32, isOutput=False)
    x_out = nc.declare_dram_parameter("x", [PER_CORE, N], f32, isOutput=True)

    with tile.TileContext(nc) as tc:
        with (
            tc.tile_pool(name="a_pool", bufs=1) as a_pool,
            tc.tile_pool(name="st_pool", bufs=1) as st_pool,
            tc.tile_pool(name="mv_pool", bufs=4, space="PSUM") as mv_pool,
            tc.tile_pool(name="pt_pool", bufs=2, space="PSUM") as pt_pool,
        ):
            ident = st_pool.tile([128, 128], f32, tag="ident")
            masks.make_identity(nc, ident[:])

            # persistent state, batch g at partition 32g
            x_t = st_pool.tile([128, N], f32, tag="x_t")
            r_t = st_pool.tile([128, N], f32, tag="r_t")
            p_t = st_pool.tile([128, N], f32, tag="p_t")
            ap_t = st_pool.tile([128, N], f32, tag="ap_t")
            b_t = st_pool.tile([128, N], f32, tag="b_t")
            tmp_t = st_pool.tile([128, N], f32, tag="tmp_t")
            wT = st_pool.tile([128, KB * 128], f32r, tag="wT")
            # scalars [128, 1]
            rr_a = st_pool.tile([128, 1], f32, tag="rr_a")
            rr_b = st_pool.tile([128, 1], f32, tag="rr_b")
            rcp_rr_a = st_pool.tile([128, 1], f32, tag="rcp_rr_a")
            rcp_rr_b = st_pool.tile([128, 1], f32, tag="rcp_rr_b")
            pap = st_pool.tile([128, 1], f32, tag="pap")
            rcp_pap = st_pool.tile([128, 1], f32, tag="rcp_pap")
            alpha = st_pool.tile([128, 1], f32, tag="alpha")
            nalpha = st_pool.tile([128, 1], f32, tag="nalpha")
            beta = st_pool.tile([128, 1], f32, tag="beta")

            for t in (x_t, r_t, p_t, ap_t, b_t, tmp_t):
                nc.vector.memset(t[:], 0.0)
            for t in (rr_a, rr_b, rcp_rr_a, rcp_rr_b, pap, rcp_pap, alpha, nalpha, beta):
                nc.vector.memset(t[:], 0.0)

            a_tiles = []
            for j in range(G):
                at = a_pool.tile([128, KB * N], f32r, tag=f"a_{j}")
                a_tiles.append(at)

            def transpose_to_wT(src):
                for kb in range(KB):
                    ps = pt_pool.tile([128, 128], f32, tag="psum_t")
                    nc.tensor.transpose(
                        ps[:], src[:, kb * 128 : (kb + 1) * 128], ident[:]
                    )
                    nc.vector.tensor_copy(wT[:, kb * 128 : (kb + 1) * 128], ps[:])

            def matvec(consume):
                """Run 4 batches' matvec; consume(j, half, psum_ap) stores it."""
                for j in range(G):
                    for half in range(2):
                        mv = mv_pool.tile([32, 512], f32, tag="mv")
                        for kb in range(KB):
                            nc.tensor.matmul(
                                mv[:, :],
                                wT[:, kb * 128 + 32 * j : kb * 128 + 32 * j + 32],
                                a_tiles[j][
                                    :, kb * N + half * 512 : kb * N + (half + 1) * 512
                                ],
                                start=(kb == 0),
                                stop=(kb == KB - 1),
                            )
                        consume(j, half, mv)

            for g in range(NGROUPS):
                # ---- load group data ----
                for j in range(G):
                    for kb in range(KB):
                        nc.sync.dma_start(
                            a_tiles[j][:, kb * N : (kb + 1) * N],
                            A_in[g * G + j, kb * 128 : (kb + 1) * 128, :],
                        )
                nc.sync.dma_start(x_t[0:128:32, :], u_in[g * G : (g + 1) * G, :])
                nc.sync.dma_start(b_t[0:128:32, :], b_in[g * G : (g + 1) * G, :])

                # ---- iter 0: r = b - A x0 ; p = r ; rr ----
                transpose_to_wT(x_t)

                def init_consume(j, half, mv):
                    nc.vector.tensor_tensor(
                        out=r_t[32 * j : 32 * j + 32, half * 512 : (half + 1) * 512],
                        in0=b_t[32 * j : 32 * j + 32, half * 512 : (half + 1) * 512],
                        in1=mv[:, :],
                        op=AT.subtract,
                    )

                matvec(init_consume)
                nc.vector.tensor_copy(p_t[:], r_t[:])
                rr_cur, rr_nxt = rr_a, rr_b
                rcp_cur, rcp_nxt = rcp_rr_a, rcp_rr_b
                nc.scalar.activation(
                    tmp_t[:], r_t[:], mybir.ActivationFunctionType.Square,
                    accum_out=rr_cur[:],
                )
                nc.vector.tensor_scalar_max(rr_cur[:], rr_cur[:], 1e-30)
                nc.vector.reciprocal(rcp_cur[:], rr_cur[:])

                # ---- CG iterations ----
                for it in range(maxiter):
                    transpose_to_wT(p_t)

                    def ap_consume(j, half, mv):
                        nc.vector.tensor_copy(
                            ap_t[32 * j : 32 * j + 32, half * 512 : (half + 1) * 512],
                            mv[:, :],
                        )

                    matvec(ap_consume)
                    # pAp
                    nc.vector.scalar_tensor_tensor(
                        out=tmp_t[:], in0=p_t[:], scalar=1.0, in1=ap_t[:],
                        op0=AT.mult, op1=AT.mult, accum_out=pap[:],
                    )
                    nc.vector.tensor_scalar_max(pap[:], pap[:], 1e-30)
                    nc.vector.reciprocal(rcp_pap[:], pap[:])
                    nc.vector.tensor_tensor(
                        out=alpha[:], in0=rr_cur[:], in1=rcp_pap[:], op=AT.mult
                    )
                    nc.vector.tensor_scalar_mul(nalpha[:], alpha[:], -1.0)
                    # x += alpha p ; r -= alpha Ap
                    nc.vector.scalar_tensor_tensor(
                        out=x_t[:], in0=p_t[:], scalar=alpha[:, 0:1], in1=x_t[:],
                        op0=AT.mult, op1=AT.add,
                    )
                    nc.vector.scalar_tensor_tensor(
                        out=r_t[:], in0=ap_t[:], scalar=nalpha[:, 0:1], in1=r_t[:],
                        op0=AT.mult, op1=AT.add,
                    )
                    # rr_new (ScalarE) ; beta = rr_new / rr
                    nc.scalar.activation(
                        tmp_t[:], r_t[:], mybir.ActivationFunctionType.Square,
                        accum_out=rr_nxt[:],
                    )
                    nc.vector.tensor_scalar_max(rr_nxt[:], rr_nxt[:], 1e-30)
                    nc.vector.tensor_tensor(
                        out=beta[:], in0=rr_nxt[:], in1=rcp_cur[:], op=AT.mult
                    )
                    nc.vector.reciprocal(rcp_nxt[:], rr_nxt[:])
                    # p = r + beta p
                    nc.vector.scalar_tensor_tensor(
                        out=p_t[:], in0=p_t[:], scalar=beta[:, 0:1], in1=r_t[:],
                        op0=AT.mult, op1=AT.add,
                    )
                    rr_cur, rr_nxt = rr_nxt, rr_cur
                    rcp_cur, rcp_nxt = rcp_nxt, rcp_cur

                # ---- store result ----
                nc.sync.dma_start(x_out[g * G : (g + 1) * G, :], x_t[0:128:32, :])

    nc.compile()
    return nc


def kernel(u, b, A, maxiter):
    maxiter = int(maxiter)
    u = np.asarray(u, dtype=np.float32)
    b = np.asarray(b, dtype=np.float32)
    A = np.asarray(A, dtype=np.float32)
    orig_shape = u.shape
    if maxiter == 0:
        return u.copy()

    from concourse.bass_utils import run_bass_kernel_spmd

    if maxiter not in _compiled_cache:
        _compiled_cache[maxiter] = _build(maxiter)
    nc = _compiled_cache[maxiter]

    u2 = u.reshape(B, N)
    b2 = b.reshape(B, N)
    in_maps = []
    for c in range(NCORES):
        s = slice(c * PER_CORE, (c + 1) * PER_CORE)
        in_maps.append({"A": A[s], "u": u2[s], "b": b2[s]})
    res = run_bass_kernel_spmd(nc, in_maps, list(range(NCORES))).results
    x = np.concatenate([res[c]["x"] for c in range(NCORES)], axis=0)
    return x.reshape(orig_shape).astype(np.float32)
